# revision 27
# baseline (speedup 1.0000x reference)
"""Trainium2 Bass kernel for nn_MetaStateStep (decay-attention + GLU block).

Sharding: 8 cores = (batch b in 0..3) x (T-half h in 0..1). Each core
processes its 1024 own rows plus a 256-row halo of following rows (the
decay weight sigmoid(3)^lag < 4e-6 beyond lag 256 contributes ~3e-5
relative output error — far under the bf16 noise floor). Fully SPMD —
one NEFF, per-core data.

The GLU down/up projections run in fp8e4m3 DoubleRow mode (2x PE rate;
the transform branch contributes ~1%% of the output magnitude, so fp8
noise lands ~1e-4 relative; weights are scaled x256 into fp8's normal
range and compensated in the norm/copy-out scales). Everything else is
bf16 (half the HBM traffic);
x and the V-contraction weights stay resident in SBUF via one fat DMA
per 128-row block (2.8KB lines — the DMA engines are descriptor-bound,
so line size matters more than transfer count). rms-norm partition sums
ride ones-stationary matmuls that broadcast the sum across every PSUM
partition. The halo slab's norm/scale chain is deferred until after the
first attention block so its scale-muls never block the score path on
the in-order vector engine. Residual adds run on DVE against PSUM.
Output is stored bf16 and upcast on host.
"""

import numpy as np
import ml_dtypes

import concourse.bass as bass
import concourse.tile as tile
from concourse import bacc, mybir
from concourse.bass_utils import run_bass_kernel_spmd
from concourse import bass_utils

# avoid artifact uploads from the trace path if a caller enables tracing
bass_utils.upload_artifacts = lambda tmpdir: "local://" + tmpdir

F32 = mybir.dt.float32
F32R = mybir.dt.float32r
BF16 = mybir.dt.bfloat16
F8 = mybir.dt.float8e4
DROW = mybir.MatmulPerfMode.DoubleRow
AF = mybir.ActivationFunctionType
NP_BF16 = ml_dtypes.bfloat16
NP_F8 = ml_dtypes.float8_e4m3fn
W8SCALE = 256.0

B, T, V = 4, 2048, 2048
D, R = 256, 512
C = 128          # chunk size
T_OWN = 1024     # rows per core
T_HALO = 256     # lookahead halo rows
T_TOT = T_OWN + T_HALO   # 1280
N_SLAB = 3               # slabs: 512, 512, 256
SLAB_W = [512, 512, T_HALO]
NVT = V // 128           # 16 v-tiles
WIN_M = 6                # score blocks per t-block (intra + 5 ahead)
WCOL = 3 * D             # w column layout: [wk | wq | wv]
EPS = float(np.finfo(np.float32).eps)

_NC_CACHE = {}


def _build_nc():
    nc = bacc.Bacc("TRN2", target_bir_lowering=False, debug=False, num_devices=8)

    xT = nc.dram_tensor("xT", [V, T_TOT], BF16, kind="ExternalInput")
    wkqvd = nc.dram_tensor("wkqvd", [V, WCOL], BF16, kind="ExternalInput")
    woT = nc.dram_tensor("woT", [D, V], BF16, kind="ExternalInput")
    wd8 = nc.dram_tensor("wd8", [NVT // 2, 128, 2 * R], F8, kind="ExternalInput")
    wu8 = nc.dram_tensor("wu8", [2, 128, 2 * V], F8, kind="ExternalInput")
    ww = nc.dram_tensor("ww", [WIN_M, C, 512], BF16, kind="ExternalInput")
    tbias_d = nc.dram_tensor("tbias", [C, R // C], F32, kind="ExternalInput")
    eps_d = nc.dram_tensor("eps", [C, 2], F32, kind="ExternalInput")
    outT = nc.dram_tensor("outT", [V, T_OWN], BF16, kind="ExternalOutput")

    with tile.TileContext(nc) as tc:
        _emit(nc, tc, xT, wkqvd, woT, wd8, wu8, ww, tbias_d, eps_d, outT)
    nc.finalize()
    return nc


def _emit(nc, tc, xT, wkqvd, woT, wd8, wu8, ww, tbias_d, eps_d, outT):
    from contextlib import ExitStack

    ctx = ExitStack()
    with ctx:
        # ---- pools (all top-level: slab2's finish interleaves phases) ----
        pers = ctx.enter_context(tc.tile_pool(name="pers", bufs=1))
        sqpool = ctx.enter_context(tc.tile_pool(name="sq", bufs=8))
        vecpool = ctx.enter_context(tc.tile_pool(name="vecs", bufs=2))
        o1pool = ctx.enter_context(tc.tile_pool(name="o1f", bufs=1))
        batt = ctx.enter_context(tc.tile_pool(name="battn", bufs=2))
        wscpool = ctx.enter_context(tc.tile_pool(name="wscp", bufs=4))
        sq2pool = ctx.enter_context(tc.tile_pool(name="sq2", bufs=3))
        vec2pool = ctx.enter_context(tc.tile_pool(name="vecs2", bufs=2))
        finpool = ctx.enter_context(tc.tile_pool(name="fin", bufs=2))
        # PSUM static budget: "a"x2 + "big"x4 + "r"x2 = 8 banks
        ps_acc = ctx.enter_context(tc.tile_pool(name="ps_acc", bufs=2, space="PSUM"))
        ps_big = ctx.enter_context(tc.tile_pool(name="ps_big", bufs=4, space="PSUM"))
        ps_r = ctx.enter_context(tc.tile_pool(name="ps_r", bufs=2, space="PSUM"))

        # ---- persistent SBUF tensors ----
        tbias_t = pers.tile([C, R // C], F32, tag="tbias")
        nc.sync.dma_start(tbias_t[:], tbias_d[:])
        eps_t = pers.tile([C, 2], F32, tag="eps")
        nc.sync.dma_start(eps_t[:], eps_d[:])
        ones_b = pers.tile([C, C], BF16, tag="ones")
        nc.vector.memset(ones_b[:], 1.0)
        ones_t = ones_b[:]

        xs, wv_t = [], []
        for vt in range(NVT):
            x = pers.tile([128, T_TOT], BF16, tag=f"xs{vt}", name=f"xs{vt}")
            nc.sync.dma_start(x[:], xT[vt * 128:(vt + 1) * 128, :])
            xs.append(x)
            w = pers.tile([128, WCOL], BF16, tag=f"w{vt}", name=f"w{vt}")
            nc.sync.dma_start(w[:], wkqvd[vt * 128:(vt + 1) * 128, :])
            wv_t.append(w)

        ww_t = []
        for m in range(WIN_M):
            w = pers.tile([C, 512], BF16, tag=f"ww{m}", name=f"ww{m}")
            nc.sync.dma_start(w[:], ww[m])
            ww_t.append(w)
        wo_t, wd8_t, wu8_t = [], [], []
        for dh in range(2):
            w = pers.tile([128, V], BF16, tag=f"wo{dh}", name=f"wo{dh}")
            nc.sync.dma_start(w[:], woT[dh * 128:(dh + 1) * 128, :])
            wo_t.append(w)
        for k in range(NVT // 2):
            w = pers.tile([128, 2, R], F8, tag=f"wd8_{k}", name=f"wd8_{k}")
            nc.sync.dma_start(w[:], wd8[k])
            wd8_t.append(w)
        for rp in range(2):
            w = pers.tile([128, 2, V], F8, tag=f"wu8_{rp}", name=f"wu8_{rp}")
            nc.sync.dma_start(w[:], wu8[rp])
            wu8_t.append(w)
        o18 = [pers.tile([128, 2, 512], F8, tag=f"o18_{k}", name=f"o18_{k}")
               for k in range(NVT // 2)]
        hg8 = [pers.tile([128, 2, 512], F8, tag=f"hg8_{rp}", name=f"hg8_{rp}")
               for rp in range(2)]

        # per-slab projection outputs
        kts = [[pers.tile([128, SLAB_W[s]], BF16, tag=f"kT{s}_{dh}",
                          name=f"kT{s}_{dh}") for dh in range(2)]
               for s in range(N_SLAB)]
        qts = [[pers.tile([128, 512], BF16, tag=f"qT{s}_{dh}", name=f"qT{s}_{dh}")
                for dh in range(2)] for s in range(2)]
        vs = [pers.tile([128, (SLAB_W[s] // 128) * D], BF16, tag=f"v{s}",
                        name=f"v{s}") for s in range(N_SLAB)]
        out1_f = [o1pool.tile([128, 512], BF16, tag=f"o1f{vt}", name=f"o1f{vt}")
                  for vt in range(NVT)]

        # =========== Phase A: q/k/v projections + norms ===========
        def emit_slab_matmuls(slab):
            t0 = slab * 512
            w = SLAB_W[slab]
            nrc = w // 128
            # ACT squares first: they gate the norm matmuls later in this
            # slab and only need the x DMAs
            sqs = []
            for vt in range(NVT):
                sq = sqpool.tile([128, 512], BF16, tag="sq")
                nc.vector.tensor_mul(sq[:, :w], xs[vt][:, t0:t0 + w],
                                     xs[vt][:, t0:t0 + w])
                sqs.append(sq)
            pks = []
            for dh in range(2):
                pk = ps_big.tile([128, 512], F32, tag="big", name=f"pk{dh}")
                for vt in range(NVT):
                    nc.tensor.matmul(pk[:, :w],
                                     wv_t[vt][:, dh * 128:(dh + 1) * 128],
                                     xs[vt][:, t0:t0 + w], start=(vt == 0),
                                     stop=(vt == NVT - 1))
                pks.append(pk)
            pqs = []
            if slab < 2:
                for dh in range(2):
                    pq = ps_big.tile([128, 512], F32, tag="big", name=f"pq{dh}")
                    for vt in range(NVT):
                        nc.tensor.matmul(pq[:],
                                         wv_t[vt][:, D + dh * 128:D + (dh + 1) * 128],
                                         xs[vt][:, t0:t0 + 512], start=(vt == 0),
                                         stop=(vt == NVT - 1))
                    pqs.append(pq)

            def emit_v(rc):
                pv = ps_acc.tile([128, D], F32, tag="a")
                for vt in range(NVT):
                    nc.tensor.matmul(pv[:],
                                     xs[vt][:, t0 + rc * 128:t0 + (rc + 1) * 128],
                                     wv_t[vt][:, 2 * D:3 * D], start=(vt == 0),
                                     stop=(vt == NVT - 1))
                nc.vector.tensor_copy(vs[slab][:, rc * D:(rc + 1) * D], pv[:])

            for rc in range(2):
                emit_v(rc)
            pn = ps_r.tile([128, 512], F32, tag="r", name="pn")
            for vt in range(NVT):
                nc.tensor.matmul(pn[:, :w], ones_t, sqs[vt][:, :w],
                                 start=(vt == 0), stop=(vt == NVT - 1))
            for rc in range(2, nrc):
                emit_v(rc)
            return pks, pqs, pn

        def emit_slab_finish(slab, pks, pqs, pn):
            w = SLAB_W[slab]
            sb1 = vecpool.tile([128, 512], F32, tag="sb1")
            nc.scalar.activation(sb1[:, :w], pn[:, :w], AF.Abs_reciprocal_sqrt,
                                 bias=eps_t[:, 0:1], scale=1.0 / V)
            sb2 = vecpool.tile([128, 512], F32, tag="sb2")
            nc.scalar.activation(sb2[:, :w], sb1[:, :w], AF.Square)
            for dh in range(2):
                nc.vector.tensor_mul(kts[slab][dh][:], pks[dh][:, :w], sb2[:, :w])
            if slab < 2:
                for dh in range(2):
                    nc.vector.tensor_mul(qts[slab][dh][:], pqs[dh][:], sb1[:])

        state = {}

        def kslice(j, dh):
            return kts[j // 4][dh][:, (j % 4) * C:(j % 4 + 1) * C]

        def emit_attn(tb):
            i0 = tb * 4
            # banded decay attention: all score blocks, then all retrieves
            # merged score+retrieve per block: the PE does 4 matmuls per
            # DVE weighting mul, so the psc ring never throttles
            pr = [ps_r.tile([128, 512], F32, tag="r", name=f"pr{dh}")
                  for dh in range(2)]
            for m in range(WIN_M):
                j = i0 + m
                psc = ps_acc.tile([128, 512], F32, tag="a")
                for dh in range(2):
                    nc.tensor.matmul(psc[:], kslice(j, dh), qts[tb][dh][:],
                                     start=(dh == 0), stop=(dh == 1))
                wsc = wscpool.tile([128, 512], BF16, tag="wsc")
                nc.vector.tensor_mul(wsc[:], psc[:], ww_t[m][:])
                for dh in range(2):
                    nc.tensor.matmul(
                        pr[dh][:],
                        vs[j // 4][:, (j % 4) * D + dh * 128:(j % 4) * D + (dh + 1) * 128],
                        wsc[:], start=(m == 0), stop=(m == WIN_M - 1))
            retr = []
            for dh in range(2):
                re = batt.tile([128, 512], BF16, tag=f"re{dh}", name=f"re{dh}")
                nc.vector.tensor_copy(re[:], pr[dh][:])
                retr.append(re)
            state[tb] = {"retr": retr}

        def emit_fused(tb):
            # Wo projection + DVE residual add + norm2 sums + down-projection;
            # down-proj/norm trail by one vt so the PE never waits on the
            # residual chain
            t0 = tb * 512
            retr = state[tb]["retr"]
            ph = [ps_big.tile([128, 512], F32, tag="big", name=f"ph{rt}")
                  for rt in range(4)]
            pn2 = ps_r.tile([128, 512], F32, tag="r", name="pn2")
            sq2s = {}

            def head(vt):
                pat = ps_acc.tile([128, 512], F32, tag="a")
                for dh in range(2):
                    nc.tensor.matmul(pat[:], wo_t[dh][:, vt * 128:(vt + 1) * 128],
                                     retr[dh][:], start=(dh == 0), stop=(dh == 1))
                nc.vector.tensor_add(out1_f[vt][:], pat[:], xs[vt][:, t0:t0 + 512])
                sq2 = sq2pool.tile([128, 512], BF16, tag="sq2")
                nc.gpsimd.tensor_mul(sq2[:], out1_f[vt][:], out1_f[vt][:])
                sq2s[vt] = sq2
                if vt < 12:
                    nc.scalar.activation(o18[vt // 2][:, vt % 2, :],
                                         out1_f[vt][:], AF.Copy)
                else:
                    nc.vector.tensor_copy(o18[vt // 2][:, vt % 2, :],
                                          out1_f[vt][:])

            def phpair(k):
                for rt in range(4):
                    nc.tensor.matmul(ph[rt][:],
                                     wd8_t[k][:, :, rt * 128:(rt + 1) * 128],
                                     o18[k][:], start=(k == 0),
                                     stop=(k == NVT // 2 - 1), perf_mode=DROW)

            def pn2mm(vt):
                nc.tensor.matmul(pn2[:], ones_t, sq2s[vt][:],
                                 start=(vt == 0), stop=(vt == NVT - 1))

            head(0)
            head(1)
            for vt in range(2, NVT):
                head(vt)
                pn2mm(vt - 2)
                if vt % 2 == 0:
                    phpair(vt // 2 - 1)
            phpair(NVT // 2 - 1)
            pn2mm(NVT - 2)
            pn2mm(NVT - 1)
            n2b = vec2pool.tile([128, 512], F32, tag="n2b")
            # wd8 is scaled x256: rsqrt(65536 * ns2) folds the 1/256 back in
            nc.scalar.activation(n2b[:], pn2[:], AF.Abs_reciprocal_sqrt,
                                 bias=eps_t[:, 1:2], scale=W8SCALE * W8SCALE / V)
            state[tb]["ph"] = ph
            state[tb]["n2b"] = n2b

        def emit_neck_head(tb):
            # norm2 scale + gelu
            ph, n2b = state[tb]["ph"], state[tb]["n2b"]
            for rt in range(4):
                hpre = batt.tile([128, 512], F32, tag="hpre")
                nc.vector.tensor_mul(hpre[:], ph[rt][:], n2b[:])
                nc.scalar.activation(hg8[rt // 2][:, rt % 2, :], hpre[:],
                                     AF.Gelu, bias=tbias_t[:, rt:rt + 1])

        def emit_neck_up(tb, vts, pool):
            # up-projection + DVE residual add + store; the pool choice
            # rides whichever PSUM ring is idle at that point
            t0 = tb * 512
            for vt in vts:
                po = pool.tile([128, 512], F32,
                               tag="big" if pool is ps_big else "a")
                for rp in range(2):
                    nc.tensor.matmul(po[:],
                                     wu8_t[rp][:, :, vt * 128:(vt + 1) * 128],
                                     hg8[rp][:], start=(rp == 0), stop=(rp == 1),
                                     perf_mode=DROW)
                fin = finpool.tile([128, 512], BF16, tag="fin")
                nc.vector.scalar_tensor_tensor(
                    fin[:], po[:], 1.0 / W8SCALE, out1_f[vt][:],
                    op0=mybir.AluOpType.mult, op1=mybir.AluOpType.add)
                nc.sync.dma_start(outT[vt * 128:(vt + 1) * 128, t0:t0 + 512],
                                  fin[:])

        # ---- schedule ----
        for slab in range(2):
            with nc.named_scope(f"slab{slab}"):
                pks, pqs, pn = emit_slab_matmuls(slab)
                emit_slab_finish(slab, pks, pqs, pn)
        with nc.named_scope("slab2"):
            s2 = emit_slab_matmuls(2)
        emit_attn(0)
        with nc.named_scope("slab2f"):
            emit_slab_finish(2, *s2)   # halo norm chain hides under attn(0)
        emit_fused(0)
        emit_attn(1)                   # covers tb0's norm2/gelu neck
        emit_neck_head(0)
        emit_neck_up(0, range(NVT), ps_big)
        emit_fused(1)
        emit_neck_head(1)
        emit_neck_up(1, range(NVT), ps_big)


def _host_prep(inputs):
    x = np.asarray(inputs["x"], dtype=np.float32)
    Wq = np.asarray(inputs["Wq"], dtype=np.float32)
    Wk = np.asarray(inputs["Wk"], dtype=np.float32)
    Wv = np.asarray(inputs["Wv"], dtype=np.float32)
    Wo = np.asarray(inputs["Wo"], dtype=np.float32)
    Wdown = np.asarray(inputs["Wdown"], dtype=np.float32)
    Wup = np.asarray(inputs["Wup"], dtype=np.float32)
    t_bias = np.asarray(inputs["t_bias"], dtype=np.float32)
    decay_logit = float(np.asarray(inputs["decay_logit"]))
    q_out_scale = float(np.asarray(inputs["q_out_scale"]))
    t_out_scale = float(np.asarray(inputs["t_out_scale"]))
    q_scale = float(np.asarray(inputs["q_scale"]).reshape(-1)[0])
    t_scale = float(np.asarray(inputs["t_scale"]).reshape(-1)[0])

    decay = 1.0 / (1.0 + np.exp(-decay_logit))

    # decay weight matrices: ww[m][ss, c*128+tt] applies to scores^T block
    # (s-chunk j = i0+m) x (t-chunk i0+c); offset o = m - c chunks.
    ww = np.zeros((WIN_M, C, 512), dtype=np.float32)
    ss = np.arange(C)[:, None].astype(np.float64)
    tt = np.arange(C)[None, :].astype(np.float64)
    for m in range(WIN_M):
        for c in range(4):
            o = m - c
            if o < 0 or o > 2:
                continue
            if o == 0:
                blk = np.where(ss > tt, decay ** (ss - tt - 1.0), 0.0)
            else:
                blk = decay ** (o * C + ss - tt - 1.0)
            ww[m, :, c * C:(c + 1) * C] = blk.astype(np.float32)

    wkqvd = np.concatenate([Wk.T, Wq.T, Wv.T], axis=1)  # [V, 3D]
    WdT = np.ascontiguousarray(Wdown.T) * np.float32(W8SCALE)
    wd8 = np.zeros((NVT // 2, 128, 2 * R), dtype=np.float32)
    for k in range(NVT // 2):
        wd8[k, :, 0:R] = WdT[(2 * k) * 128:(2 * k + 1) * 128, :]
        wd8[k, :, R:2 * R] = WdT[(2 * k + 1) * 128:(2 * k + 2) * 128, :]
    WuT = (np.ascontiguousarray(Wup.T)
           * np.float32(t_scale * t_out_scale * W8SCALE))  # [R, V]
    wu8 = np.zeros((2, 128, 2 * V), dtype=np.float32)
    for rp in range(2):
        wu8[rp, :, 0:V] = WuT[(2 * rp) * 128:(2 * rp + 1) * 128, :]
        wu8[rp, :, V:2 * V] = WuT[(2 * rp + 1) * 128:(2 * rp + 2) * 128, :]

    shared = {
        "wkqvd": np.ascontiguousarray(wkqvd).astype(NP_BF16),
        "woT": (np.ascontiguousarray(Wo.T)
                * np.float32(q_scale * q_out_scale)).astype(NP_BF16),
        "wd8": wd8.astype(NP_F8),
        "wu8": wu8.astype(NP_F8),
        "ww": ww.astype(NP_BF16),
        "tbias": np.ascontiguousarray(t_bias.reshape(R // C, C).T),
        "eps": np.stack([np.full(C, EPS, np.float32),
                         np.full(C, EPS * W8SCALE * W8SCALE, np.float32)],
                        axis=1),
    }

    in_maps = []
    for core in range(8):
        b, h = core // 2, core % 2
        own = x[b, h * T_OWN:(h + 1) * T_OWN, :]
        if h == 0:
            halo = x[b, T_OWN:T_OWN + T_HALO, :]
        else:
            halo = np.zeros((T_HALO, V), np.float32)
        xT_c = np.ascontiguousarray(
            np.concatenate([own, halo], axis=0).T).astype(NP_BF16)
        m = dict(shared)
        m["xT"] = xT_c
        in_maps.append(m)
    return in_maps


def kernel(**inputs) -> np.ndarray:
    if "nc" not in _NC_CACHE:
        _NC_CACHE["nc"] = _build_nc()
    nc = _NC_CACHE["nc"]
    in_maps = _host_prep(inputs)
    res = run_bass_kernel_spmd(nc, in_maps, core_ids=list(range(8)))
    out = np.empty((B, T, V), np.float32)
    for core in range(8):
        b, h = core // 2, core % 2
        out[b, h * T_OWN:(h + 1) * T_OWN, :] = \
            res.results[core]["outT"].astype(np.float32).T
    return out


# revision 28
# speedup vs baseline: 1.1793x; 1.1793x over previous
"""Trainium2 Bass kernel for nn_MetaStateStep (decay-attention + GLU block).

Sharding: 8 cores = (batch b in 0..3) x (T-half h in 0..1). Each core
processes its 1024 own rows plus a 256-row halo of following rows (the
decay weight sigmoid(3)^lag < 4e-6 beyond lag 256 contributes ~3e-5
relative output error — far under the bf16 noise floor). Fully SPMD —
one NEFF, per-core data.

The GLU down/up projections run in fp8e4m3 DoubleRow mode (2x PE rate;
the transform branch contributes ~1%% of the output magnitude, so fp8
noise lands ~1e-4 relative; weights are scaled x256 into fp8's normal
range and compensated in the norm/copy-out scales). Everything else is
bf16 (half the HBM traffic);
x and the V-contraction weights stay resident in SBUF via one fat DMA
per 128-row block (2.8KB lines — the DMA engines are descriptor-bound,
so line size matters more than transfer count). rms-norm partition sums
ride ones-stationary matmuls that broadcast the sum across every PSUM
partition. The halo slab's norm/scale chain is deferred until after the
first attention block so its scale-muls never block the score path on
the in-order vector engine. Residual adds run on DVE against PSUM.
Output is stored bf16 and upcast on host.
"""

import numpy as np
import ml_dtypes

import concourse.bass as bass
import concourse.tile as tile
from concourse import bacc, mybir
from concourse.bass_utils import run_bass_kernel_spmd
from concourse import bass_utils

# avoid artifact uploads from the trace path if a caller enables tracing
bass_utils.upload_artifacts = lambda tmpdir: "local://" + tmpdir

F32 = mybir.dt.float32
F32R = mybir.dt.float32r
BF16 = mybir.dt.bfloat16
F8 = mybir.dt.float8e4
DROW = mybir.MatmulPerfMode.DoubleRow
AF = mybir.ActivationFunctionType
NP_BF16 = ml_dtypes.bfloat16
NP_F8 = ml_dtypes.float8_e4m3fn
W8SCALE = 256.0

B, T, V = 4, 2048, 2048
D, R = 256, 512
C = 128          # chunk size
T_OWN = 1024     # rows per core
T_HALO = 256     # lookahead halo rows
T_TOT = T_OWN + T_HALO   # 1280
N_SLAB = 3               # slabs: 512, 512, 256
SLAB_W = [512, 512, T_HALO]
NVT = V // 128           # 16 v-tiles
WIN_M = 6                # score blocks per t-block (intra + 5 ahead)
WCOL = 3 * D             # w column layout: [wk | wq | wv]
EPS = float(np.finfo(np.float32).eps)

_NC_CACHE = {}


def _build_nc():
    nc = bacc.Bacc("TRN2", target_bir_lowering=False, debug=False, num_devices=8)

    xT = nc.dram_tensor("xT", [V, T_TOT], BF16, kind="ExternalInput")
    wkqvd = nc.dram_tensor("wkqvd", [V, WCOL], BF16, kind="ExternalInput")
    woT = nc.dram_tensor("woT", [D, V], BF16, kind="ExternalInput")
    wd8 = nc.dram_tensor("wd8", [NVT // 2, 128, 2 * R], F8, kind="ExternalInput")
    wu8 = nc.dram_tensor("wu8", [2, 128, 2 * V], F8, kind="ExternalInput")
    ww = nc.dram_tensor("ww", [WIN_M, C, 512], BF16, kind="ExternalInput")
    tbias_d = nc.dram_tensor("tbias", [C, R // C], F32, kind="ExternalInput")
    eps_d = nc.dram_tensor("eps", [C, 2], F32, kind="ExternalInput")
    outT = nc.dram_tensor("outT", [V, T_OWN], BF16, kind="ExternalOutput")

    with tile.TileContext(nc) as tc:
        _emit(nc, tc, xT, wkqvd, woT, wd8, wu8, ww, tbias_d, eps_d, outT)
    nc.finalize()
    return nc


def _emit(nc, tc, xT, wkqvd, woT, wd8, wu8, ww, tbias_d, eps_d, outT):
    from contextlib import ExitStack

    ctx = ExitStack()
    with ctx:
        # ---- pools (all top-level: slab2's finish interleaves phases) ----
        pers = ctx.enter_context(tc.tile_pool(name="pers", bufs=1))
        sqpool = ctx.enter_context(tc.tile_pool(name="sq", bufs=8))
        vecpool = ctx.enter_context(tc.tile_pool(name="vecs", bufs=2))
        o1pool = ctx.enter_context(tc.tile_pool(name="o1f", bufs=1))
        batt = ctx.enter_context(tc.tile_pool(name="battn", bufs=2))
        wscpool = ctx.enter_context(tc.tile_pool(name="wscp", bufs=4))
        sq2pool = ctx.enter_context(tc.tile_pool(name="sq2", bufs=3))
        vec2pool = ctx.enter_context(tc.tile_pool(name="vecs2", bufs=2))
        finpool = ctx.enter_context(tc.tile_pool(name="fin", bufs=2))
        # PSUM static budget: "a"x2 + "big"x4 + "r"x2 = 8 banks
        ps_acc = ctx.enter_context(tc.tile_pool(name="ps_acc", bufs=2, space="PSUM"))
        ps_big = ctx.enter_context(tc.tile_pool(name="ps_big", bufs=4, space="PSUM"))
        ps_r = ctx.enter_context(tc.tile_pool(name="ps_r", bufs=2, space="PSUM"))

        # ---- persistent SBUF tensors ----
        tbias_t = pers.tile([C, R // C], F32, tag="tbias")
        nc.sync.dma_start(tbias_t[:], tbias_d[:])
        eps_t = pers.tile([C, 2], F32, tag="eps")
        nc.sync.dma_start(eps_t[:], eps_d[:])
        ones_b = pers.tile([C, C], BF16, tag="ones")
        nc.vector.memset(ones_b[:], 1.0)
        ones_t = ones_b[:]

        xs, wv_t = [], []
        for vt in range(NVT):
            x = pers.tile([128, T_TOT], BF16, tag=f"xs{vt}", name=f"xs{vt}")
            nc.sync.dma_start(x[:], xT[vt * 128:(vt + 1) * 128, :])
            xs.append(x)
            w = pers.tile([128, WCOL], BF16, tag=f"w{vt}", name=f"w{vt}")
            nc.sync.dma_start(w[:], wkqvd[vt * 128:(vt + 1) * 128, :])
            wv_t.append(w)

        ww_t = []
        for m in range(WIN_M):
            w = pers.tile([C, 512], BF16, tag=f"ww{m}", name=f"ww{m}")
            nc.sync.dma_start(w[:], ww[m])
            ww_t.append(w)
        wo_t, wd8_t, wu8_t = [], [], []
        for dh in range(2):
            w = pers.tile([128, V], BF16, tag=f"wo{dh}", name=f"wo{dh}")
            nc.sync.dma_start(w[:], woT[dh * 128:(dh + 1) * 128, :])
            wo_t.append(w)
        for k in range(NVT // 2):
            w = pers.tile([128, 2, R], F8, tag=f"wd8_{k}", name=f"wd8_{k}")
            nc.sync.dma_start(w[:], wd8[k])
            wd8_t.append(w)
        for rp in range(2):
            w = pers.tile([128, 2, V], F8, tag=f"wu8_{rp}", name=f"wu8_{rp}")
            nc.sync.dma_start(w[:], wu8[rp])
            wu8_t.append(w)
        o18 = [pers.tile([128, 2, 512], F8, tag=f"o18_{k}", name=f"o18_{k}")
               for k in range(NVT // 2)]
        hg8 = [pers.tile([128, 2, 512], F8, tag=f"hg8_{rp}", name=f"hg8_{rp}")
               for rp in range(2)]

        # per-slab projection outputs
        kts = [[pers.tile([128, SLAB_W[s]], BF16, tag=f"kT{s}_{dh}",
                          name=f"kT{s}_{dh}") for dh in range(2)]
               for s in range(N_SLAB)]
        qts = [[pers.tile([128, 512], BF16, tag=f"qT{s}_{dh}", name=f"qT{s}_{dh}")
                for dh in range(2)] for s in range(2)]
        vs = [pers.tile([128, (SLAB_W[s] // 128) * D], BF16, tag=f"v{s}",
                        name=f"v{s}") for s in range(N_SLAB)]
        out1_f = [o1pool.tile([128, 512], BF16, tag=f"o1f{vt}", name=f"o1f{vt}")
                  for vt in range(NVT)]

        # =========== Phase A: q/k/v projections + norms ===========
        def emit_slab_matmuls(slab):
            t0 = slab * 512
            w = SLAB_W[slab]
            nrc = w // 128
            # ACT squares first: they gate the norm matmuls later in this
            # slab and only need the x DMAs
            sqs = []
            for vt in range(NVT):
                sq = sqpool.tile([128, 512], BF16, tag="sq")
                nc.vector.tensor_mul(sq[:, :w], xs[vt][:, t0:t0 + w],
                                     xs[vt][:, t0:t0 + w])
                sqs.append(sq)
            pks = []
            for dh in range(2):
                pk = ps_big.tile([128, 512], F32, tag="big", name=f"pk{dh}")
                for vt in range(NVT):
                    nc.tensor.matmul(pk[:, :w],
                                     wv_t[vt][:, dh * 128:(dh + 1) * 128],
                                     xs[vt][:, t0:t0 + w], start=(vt == 0),
                                     stop=(vt == NVT - 1))
                pks.append(pk)
            pqs = []
            if slab < 2:
                for dh in range(2):
                    pq = ps_big.tile([128, 512], F32, tag="big", name=f"pq{dh}")
                    for vt in range(NVT):
                        nc.tensor.matmul(pq[:],
                                         wv_t[vt][:, D + dh * 128:D + (dh + 1) * 128],
                                         xs[vt][:, t0:t0 + 512], start=(vt == 0),
                                         stop=(vt == NVT - 1))
                    pqs.append(pq)

            def emit_v(rc):
                pv = ps_acc.tile([128, D], F32, tag="a")
                for vt in range(NVT):
                    nc.tensor.matmul(pv[:],
                                     xs[vt][:, t0 + rc * 128:t0 + (rc + 1) * 128],
                                     wv_t[vt][:, 2 * D:3 * D], start=(vt == 0),
                                     stop=(vt == NVT - 1))
                nc.vector.tensor_copy(vs[slab][:, rc * D:(rc + 1) * D], pv[:])

            for rc in range(2):
                emit_v(rc)
            pn = ps_r.tile([128, 512], F32, tag="r", name="pn")
            for vt in range(NVT):
                nc.tensor.matmul(pn[:, :w], ones_t, sqs[vt][:, :w],
                                 start=(vt == 0), stop=(vt == NVT - 1))
            for rc in range(2, nrc):
                emit_v(rc)
            return pks, pqs, pn

        def emit_slab_finish(slab, pks, pqs, pn):
            w = SLAB_W[slab]
            sb1 = vecpool.tile([128, 512], F32, tag="sb1")
            nc.scalar.activation(sb1[:, :w], pn[:, :w], AF.Abs_reciprocal_sqrt,
                                 bias=eps_t[:, 0:1], scale=1.0 / V)
            sb2 = vecpool.tile([128, 512], F32, tag="sb2")
            nc.scalar.activation(sb2[:, :w], sb1[:, :w], AF.Square)
            for dh in range(2):
                nc.vector.tensor_mul(kts[slab][dh][:], pks[dh][:, :w], sb2[:, :w])
            if slab < 2:
                for dh in range(2):
                    nc.vector.tensor_mul(qts[slab][dh][:], pqs[dh][:], sb1[:])

        state = {}

        def kslice(j, dh):
            return kts[j // 4][dh][:, (j % 4) * C:(j % 4 + 1) * C]

        def emit_attn(tb):
            i0 = tb * 4
            # banded decay attention: all score blocks, then all retrieves
            # merged score+retrieve per block: the PE does 4 matmuls per
            # DVE weighting mul, so the psc ring never throttles
            pr = [ps_r.tile([128, 512], F32, tag="r", name=f"pr{dh}")
                  for dh in range(2)]
            for m in range(WIN_M):
                j = i0 + m
                psc = ps_acc.tile([128, 512], F32, tag="a")
                for dh in range(2):
                    nc.tensor.matmul(psc[:], kslice(j, dh), qts[tb][dh][:],
                                     start=(dh == 0), stop=(dh == 1))
                wsc = wscpool.tile([128, 512], BF16, tag="wsc")
                nc.vector.tensor_mul(wsc[:], psc[:], ww_t[m][:])
                for dh in range(2):
                    nc.tensor.matmul(
                        pr[dh][:],
                        vs[j // 4][:, (j % 4) * D + dh * 128:(j % 4) * D + (dh + 1) * 128],
                        wsc[:], start=(m == 0), stop=(m == WIN_M - 1))
            retr = []
            for dh in range(2):
                re = batt.tile([128, 512], BF16, tag=f"re{dh}", name=f"re{dh}")
                nc.vector.tensor_copy(re[:], pr[dh][:])
                retr.append(re)
            state[tb] = {"retr": retr}

        def emit_fused(tb):
            # Wo projection + DVE residual add + norm2 sums + down-projection;
            # down-proj/norm trail by one vt so the PE never waits on the
            # residual chain
            t0 = tb * 512
            retr = state[tb]["retr"]
            ph = [ps_big.tile([128, 512], F32, tag="big", name=f"ph{rt}")
                  for rt in range(4)]
            pn2 = ps_r.tile([128, 512], F32, tag="r", name="pn2")
            sq2s = {}

            def head(vt):
                pat = ps_acc.tile([128, 512], F32, tag="a")
                for dh in range(2):
                    nc.tensor.matmul(pat[:], wo_t[dh][:, vt * 128:(vt + 1) * 128],
                                     retr[dh][:], start=(dh == 0), stop=(dh == 1))
                nc.vector.tensor_add(out1_f[vt][:], pat[:], xs[vt][:, t0:t0 + 512])
                sq2 = sq2pool.tile([128, 512], BF16, tag="sq2")
                nc.vector.tensor_mul(sq2[:], out1_f[vt][:], out1_f[vt][:])
                sq2s[vt] = sq2
                if vt < 12:
                    nc.scalar.activation(o18[vt // 2][:, vt % 2, :],
                                         out1_f[vt][:], AF.Copy)
                else:
                    nc.vector.tensor_copy(o18[vt // 2][:, vt % 2, :],
                                          out1_f[vt][:])

            def phpair(k):
                for rt in range(4):
                    nc.tensor.matmul(ph[rt][:],
                                     wd8_t[k][:, :, rt * 128:(rt + 1) * 128],
                                     o18[k][:], start=(k == 0),
                                     stop=(k == NVT // 2 - 1), perf_mode=DROW)

            def pn2mm(vt):
                nc.tensor.matmul(pn2[:], ones_t, sq2s[vt][:],
                                 start=(vt == 0), stop=(vt == NVT - 1))

            head(0)
            head(1)
            for vt in range(2, NVT):
                head(vt)
                pn2mm(vt - 2)
                if vt % 2 == 0:
                    phpair(vt // 2 - 1)
            phpair(NVT // 2 - 1)
            pn2mm(NVT - 2)
            pn2mm(NVT - 1)
            n2b = vec2pool.tile([128, 512], F32, tag="n2b")
            # wd8 is scaled x256: rsqrt(65536 * ns2) folds the 1/256 back in
            nc.scalar.activation(n2b[:], pn2[:], AF.Abs_reciprocal_sqrt,
                                 bias=eps_t[:, 1:2], scale=W8SCALE * W8SCALE / V)
            state[tb]["ph"] = ph
            state[tb]["n2b"] = n2b

        def emit_neck_head(tb):
            # norm2 scale + gelu
            ph, n2b = state[tb]["ph"], state[tb]["n2b"]
            for rt in range(4):
                hpre = batt.tile([128, 512], F32, tag="hpre")
                nc.vector.tensor_mul(hpre[:], ph[rt][:], n2b[:])
                nc.scalar.activation(hg8[rt // 2][:, rt % 2, :], hpre[:],
                                     AF.Gelu, bias=tbias_t[:, rt:rt + 1])

        def emit_neck_up(tb, vts, pool):
            # up-projection + DVE residual add + store; the pool choice
            # rides whichever PSUM ring is idle at that point
            t0 = tb * 512
            for vt in vts:
                po = pool.tile([128, 512], F32,
                               tag="big" if pool is ps_big else "a")
                for rp in range(2):
                    nc.tensor.matmul(po[:],
                                     wu8_t[rp][:, :, vt * 128:(vt + 1) * 128],
                                     hg8[rp][:], start=(rp == 0), stop=(rp == 1),
                                     perf_mode=DROW)
                fin = finpool.tile([128, 512], BF16, tag="fin")
                nc.vector.scalar_tensor_tensor(
                    fin[:], po[:], 1.0 / W8SCALE, out1_f[vt][:],
                    op0=mybir.AluOpType.mult, op1=mybir.AluOpType.add)
                nc.sync.dma_start(outT[vt * 128:(vt + 1) * 128, t0:t0 + 512],
                                  fin[:])

        # ---- schedule ----
        for slab in range(2):
            with nc.named_scope(f"slab{slab}"):
                pks, pqs, pn = emit_slab_matmuls(slab)
                emit_slab_finish(slab, pks, pqs, pn)
        with nc.named_scope("slab2"):
            s2 = emit_slab_matmuls(2)
        emit_attn(0)
        with nc.named_scope("slab2f"):
            emit_slab_finish(2, *s2)   # halo norm chain hides under attn(0)
        emit_fused(0)
        emit_attn(1)                   # covers tb0's norm2/gelu neck
        emit_neck_head(0)
        emit_neck_up(0, range(NVT), ps_big)
        emit_fused(1)
        emit_neck_head(1)
        emit_neck_up(1, range(NVT), ps_big)


def _host_prep(inputs):
    x = np.asarray(inputs["x"], dtype=np.float32)
    Wq = np.asarray(inputs["Wq"], dtype=np.float32)
    Wk = np.asarray(inputs["Wk"], dtype=np.float32)
    Wv = np.asarray(inputs["Wv"], dtype=np.float32)
    Wo = np.asarray(inputs["Wo"], dtype=np.float32)
    Wdown = np.asarray(inputs["Wdown"], dtype=np.float32)
    Wup = np.asarray(inputs["Wup"], dtype=np.float32)
    t_bias = np.asarray(inputs["t_bias"], dtype=np.float32)
    decay_logit = float(np.asarray(inputs["decay_logit"]))
    q_out_scale = float(np.asarray(inputs["q_out_scale"]))
    t_out_scale = float(np.asarray(inputs["t_out_scale"]))
    q_scale = float(np.asarray(inputs["q_scale"]).reshape(-1)[0])
    t_scale = float(np.asarray(inputs["t_scale"]).reshape(-1)[0])

    decay = 1.0 / (1.0 + np.exp(-decay_logit))

    # decay weight matrices: ww[m][ss, c*128+tt] applies to scores^T block
    # (s-chunk j = i0+m) x (t-chunk i0+c); offset o = m - c chunks.
    ww = np.zeros((WIN_M, C, 512), dtype=np.float32)
    ss = np.arange(C)[:, None].astype(np.float64)
    tt = np.arange(C)[None, :].astype(np.float64)
    for m in range(WIN_M):
        for c in range(4):
            o = m - c
            if o < 0 or o > 2:
                continue
            if o == 0:
                blk = np.where(ss > tt, decay ** (ss - tt - 1.0), 0.0)
            else:
                blk = decay ** (o * C + ss - tt - 1.0)
            ww[m, :, c * C:(c + 1) * C] = blk.astype(np.float32)

    wkqvd = np.concatenate([Wk.T, Wq.T, Wv.T], axis=1)  # [V, 3D]
    WdT = np.ascontiguousarray(Wdown.T) * np.float32(W8SCALE)
    wd8 = np.zeros((NVT // 2, 128, 2 * R), dtype=np.float32)
    for k in range(NVT // 2):
        wd8[k, :, 0:R] = WdT[(2 * k) * 128:(2 * k + 1) * 128, :]
        wd8[k, :, R:2 * R] = WdT[(2 * k + 1) * 128:(2 * k + 2) * 128, :]
    WuT = (np.ascontiguousarray(Wup.T)
           * np.float32(t_scale * t_out_scale * W8SCALE))  # [R, V]
    wu8 = np.zeros((2, 128, 2 * V), dtype=np.float32)
    for rp in range(2):
        wu8[rp, :, 0:V] = WuT[(2 * rp) * 128:(2 * rp + 1) * 128, :]
        wu8[rp, :, V:2 * V] = WuT[(2 * rp + 1) * 128:(2 * rp + 2) * 128, :]

    shared = {
        "wkqvd": np.ascontiguousarray(wkqvd).astype(NP_BF16),
        "woT": (np.ascontiguousarray(Wo.T)
                * np.float32(q_scale * q_out_scale)).astype(NP_BF16),
        "wd8": wd8.astype(NP_F8),
        "wu8": wu8.astype(NP_F8),
        "ww": ww.astype(NP_BF16),
        "tbias": np.ascontiguousarray(t_bias.reshape(R // C, C).T),
        "eps": np.stack([np.full(C, EPS, np.float32),
                         np.full(C, EPS * W8SCALE * W8SCALE, np.float32)],
                        axis=1),
    }

    in_maps = []
    for core in range(8):
        b, h = core // 2, core % 2
        own = x[b, h * T_OWN:(h + 1) * T_OWN, :]
        if h == 0:
            halo = x[b, T_OWN:T_OWN + T_HALO, :]
        else:
            halo = np.zeros((T_HALO, V), np.float32)
        xT_c = np.ascontiguousarray(
            np.concatenate([own, halo], axis=0).T).astype(NP_BF16)
        m = dict(shared)
        m["xT"] = xT_c
        in_maps.append(m)
    return in_maps


def kernel(**inputs) -> np.ndarray:
    if "nc" not in _NC_CACHE:
        _NC_CACHE["nc"] = _build_nc()
    nc = _NC_CACHE["nc"]
    in_maps = _host_prep(inputs)
    res = run_bass_kernel_spmd(nc, in_maps, core_ids=list(range(8)))
    out = np.empty((B, T, V), np.float32)
    for core in range(8):
        b, h = core // 2, core % 2
        out[b, h * T_OWN:(h + 1) * T_OWN, :] = \
            res.results[core]["outT"].astype(np.float32).T
    return out
